# revision 1
# baseline (speedup 1.0000x reference)
"""Trainium2 Bass kernel for a 4-term video/query contrastive loss.

Strategy: data-parallel over batch B=64 across 8 cores (8 videos/core).
The dominant work is contrasting every query / top-k feature against every
upper-triangular 2d-map proposal feature of every video:

  - host compacts video_feats to the 2080 triu columns (padded to 2560)
    and casts to bf16; weights = [64 queries | 16 local topk feats],
    L2-normalized on host, also bf16
  - scores are computed TRANSPOSED, S^T[p, r] (proposals on partitions),
    so the per-proposal 1/||v_p|| becomes a per-partition scale that the
    scalar engine applies inside exp() for free
  - per-proposal squared norms: matmul of V*V against a ones column
  - exp sums and the two iou-masked sums come from one tiny N=3 matmul
    per chunk with rhs = [valid_mask | pos_mask | neg_mask], accumulated
    in PSUM across each video's 20 chunks
  - the kernel is emitted in two phases (all Sqrt, then all Exp) so the
    scalar engine loads each activation LUT exactly once

Host (numpy) does index prep (top-k gather, masks, normalization of the
80 weight rows) and final log/mean assembly over ~10^3 scalars.
"""

import numpy as np
import ml_dtypes

import concourse.bacc as bacc
import concourse.bass as bass
import concourse.tile as tile
from concourse import mybir
from concourse import bass_utils

f32 = mybir.dt.float32
bf16 = mybir.dt.bfloat16
AOT = mybir.AluOpType
AFT = mybir.ActivationFunctionType
BF = ml_dtypes.bfloat16

B, C, D = 64, 256, 64
SP = D * D                 # 4096 flattened 2d-map positions
NTRIU = D * (D + 1) // 2   # 2080 upper-tri positions
SPP = 2560                 # compacted + padded proposal count (20 x 128)
NCORES = 8
VB = B // NCORES           # videos per core: 8
NPT = 2                    # sentences (num_targets) per video
NTC = NPT * VB             # sentences per core: 16
M80 = B + NTC              # score rows: 64 queries + 16 local topk feats
T = B * NPT                # 128 sentences
SMW = B + NTC + T          # 208 small-feature columns
PCH = 128                  # proposals per chunk (partition dim of S^T)
NPC = SPP // PCH           # 20 chunks per video
TAU_I = 10.0               # 1/temperature (T_V == T_Q == 0.1)
NEG_IOU = 0.5


def _build_module():
    nc = bacc.Bacc("TRN2", target_bir_lowering=False, debug=False)

    d_v = nc.dram_tensor("v16", (VB * C, SPP), bf16, kind="ExternalInput")
    d_w = nc.dram_tensor("w16", (C, M80), bf16, kind="ExternalInput")
    d_sf = nc.dram_tensor("sf", (C, SMW), f32, kind="ExternalInput")
    d_msk = nc.dram_tensor("msk", (VB * PCH, NPC * 3), bf16, kind="ExternalInput")
    d_bd = nc.dram_tensor("bd", (B, T), f32, kind="ExternalInput")
    d_qrn = nc.dram_tensor("qrn", (B, 1), f32, kind="ExternalInput")
    d_trn = nc.dram_tensor("trn", (NTC, 1), f32, kind="ExternalInput")

    d_osmall = nc.dram_tensor("o_small", (B, NTC + T), f32, kind="ExternalOutput")
    d_oa3 = nc.dram_tensor("o_a3", (NTC, NTC), f32, kind="ExternalOutput")
    d_ocs1 = nc.dram_tensor("o_cs1", (1, NTC), f32, kind="ExternalOutput")
    d_os4 = nc.dram_tensor("o_s4", (B, 2), f32, kind="ExternalOutput")
    d_or = nc.dram_tensor("o_r", (M80, VB * 3), f32, kind="ExternalOutput")

    with tile.TileContext(nc) as tc:
        with (
            tc.tile_pool(name="consts", bufs=1) as cp,
            tc.tile_pool(name="smalls", bufs=1) as sm,
            tc.tile_pool(name="sq", bufs=4) as sqp,
            tc.tile_pool(name="etile", bufs=4) as ep,
            tc.tile_pool(name="outs", bufs=1) as op_,
            tc.tile_pool(name="pn", bufs=2, space="PSUM") as pn,
            tc.tile_pool(name="ps", bufs=4, space="PSUM") as ps,
            tc.tile_pool(name="pr", bufs=2, space="PSUM") as pr,
        ):
            # ---- resident inputs ----
            vts = []
            for v in range(VB):
                for k in range(2):
                    vt = cp.tile([128, SPP], bf16, tag=f"v{v}_{k}")
                    nc.sync.dma_start(vt, d_v[v * C + k * 128:v * C + (k + 1) * 128, :])
                    vts.append(vt)
            w0 = cp.tile([128, M80], bf16, tag="w0")
            w1 = cp.tile([128, M80], bf16, tag="w1")
            nc.sync.dma_start(w0, d_w[0:128, :])
            nc.sync.dma_start(w1, d_w[128:256, :])
            sf0 = cp.tile([128, SMW], f32, tag="sf0")
            sf1 = cp.tile([128, SMW], f32, tag="sf1")
            nc.sync.dma_start(sf0, d_sf[0:128, :])
            nc.sync.dma_start(sf1, d_sf[128:256, :])
            msk_t = cp.tile([PCH, VB * NPC * 3], bf16, tag="msk")
            for v in range(VB):
                nc.sync.dma_start(msk_t[:, v * NPC * 3:(v + 1) * NPC * 3],
                                  d_msk[v * PCH:(v + 1) * PCH, :])
            bd_t = cp.tile([B, T], f32, tag="bd")
            nc.sync.dma_start(bd_t, d_bd[:])
            qrn = cp.tile([B, 1], f32, tag="qrn")
            nc.sync.dma_start(qrn, d_qrn[:])
            trn = cp.tile([NTC, 1], f32, tag="trn")
            nc.sync.dma_start(trn, d_trn[:])
            ones_t = cp.tile([128, 1], f32, tag="ones")
            nc.vector.memset(ones_t, 1.0)
            ones16 = cp.tile([128, 1], bf16, tag="ones16")
            nc.vector.memset(ones16, 1.0)
            ones80 = cp.tile([1, M80], f32, tag="ones80")
            nc.vector.memset(ones80, 1.0)
            rm10 = cp.tile([128, VB * NPC], f32, tag="rm10")

            # ---- small phase, part 1 (ACT: Sqrt/Copy table) ----
            sqf = sm.tile([128, SMW], f32, tag="sqf")
            nps = pn.tile([1, SMW], f32, tag="np")
            nc.vector.tensor_mul(sqf, sf0, sf0)
            nc.tensor.matmul(nps, ones_t, sqf, start=True, stop=False)
            sqf2 = sm.tile([128, SMW], f32, tag="sqf2")
            nc.vector.tensor_mul(sqf2, sf1, sf1)
            nc.tensor.matmul(nps, ones_t, sqf2, start=False, stop=True)
            nsr = sm.tile([1, SMW], f32, tag="nsr")
            nc.scalar.sqrt(nsr, nps)
            nc.vector.tensor_scalar_max(nsr, nsr, 1e-12)
            rvec = sm.tile([1, SMW], f32, tag="rvec")
            nc.vector.reciprocal(rvec, nsr)
            # broadcast rvec cols [B:SMW] to 64 partitions via K=1 matmul
            rvb_ps = pr.tile([B, NTC + T], f32, tag="racc")
            nc.tensor.matmul(rvb_ps, ones80[:, 0:B], rvec[0:1, B:SMW],
                             start=True, stop=True)
            rvb = sm.tile([B, NTC + T], f32, tag="rvb")
            nc.scalar.copy(rvb, rvb_ps)

            # ---- big phase A: per-proposal 1/norms (ACT: Sqrt) ----
            for v in range(VB):
                for c in range(NPC):
                    sl = slice(c * PCH, (c + 1) * PCH)
                    col = v * NPC + c
                    sq0 = sqp.tile([128, PCH], bf16, tag="sq")
                    nc.vector.tensor_mul(sq0, vts[2 * v][:, sl], vts[2 * v][:, sl])
                    np_ = pn.tile([128, 1], f32, tag="np")
                    nc.tensor.matmul(np_, sq0, ones16, start=True, stop=False)
                    sq1 = sqp.tile([128, PCH], bf16, tag="sq")
                    nc.vector.tensor_mul(sq1, vts[2 * v + 1][:, sl],
                                         vts[2 * v + 1][:, sl])
                    nc.tensor.matmul(np_, sq1, ones16, start=False, stop=True)
                    nr = sqp.tile([128, 1], f32, tag="nr")
                    nc.scalar.sqrt(nr, np_)
                    nc.vector.tensor_scalar_max(nr, nr, 1e-12)
                    nc.vector.reciprocal(rm10[:, col:col + 1], nr)
            nc.vector.tensor_scalar_mul(rm10, rm10, TAU_I)

            # ---- small phase, part 2 (ACT: Exp; before phase B) ----
            aps_ = ps.tile([B, NTC + T], f32, tag="st")
            nc.tensor.matmul(aps_, sf0[:, 0:B], sf0[:, B:SMW], start=True, stop=False)
            nc.tensor.matmul(aps_, sf1[:, 0:B], sf1[:, B:SMW], start=False, stop=True)
            nc.vector.tensor_scalar(out=aps_, in0=aps_, scalar1=qrn,
                                    scalar2=None, op0=AOT.mult)
            smalls = sm.tile([B, NTC + T], f32, tag="smalls")
            nc.vector.tensor_mul(smalls, aps_, rvb)
            nc.gpsimd.dma_start(d_osmall[:], smalls)

            a3ps = ps.tile([NTC, NTC], f32, tag="st")
            nc.tensor.matmul(a3ps, sf0[:, B:B + NTC], sf0[:, B:B + NTC],
                             start=True, stop=False)
            nc.tensor.matmul(a3ps, sf1[:, B:B + NTC], sf1[:, B:B + NTC],
                             start=False, stop=True)
            nc.vector.tensor_scalar(out=a3ps, in0=a3ps, scalar1=trn,
                                    scalar2=None, op0=AOT.mult)
            a3s = sm.tile([NTC, NTC], f32, tag="a3s")
            nc.vector.tensor_mul(a3s, a3ps, rvb[0:NTC, 0:NTC])
            nc.gpsimd.dma_start(d_oa3[:], a3s)

            e1 = sm.tile([B, NTC], f32, tag="e1")
            nc.scalar.activation(e1, smalls[:, 0:NTC], AFT.Exp, scale=TAU_I)
            cs1ps = pn.tile([1, NTC], f32, tag="np")
            nc.tensor.matmul(cs1ps, ones_t[0:B, :], e1, start=True, stop=True)
            cs1s = sm.tile([1, NTC], f32, tag="cs1s")
            nc.vector.tensor_copy(cs1s, cs1ps)
            nc.gpsimd.dma_start(d_ocs1[:], cs1s)

            e4 = sm.tile([B, T], f32, tag="e4")
            s4o = sm.tile([B, 2], f32, tag="s4o")
            nc.scalar.activation(e4, smalls[:, NTC:], AFT.Exp, scale=TAU_I,
                                 accum_out=s4o[:, 0:1])
            nc.vector.tensor_mul(e4, e4, bd_t)
            nc.vector.tensor_reduce(s4o[:, 1:2], e4, mybir.AxisListType.X, AOT.add)
            nc.gpsimd.dma_start(d_os4[:], s4o)

            # ---- big phase B: scores, exp, masked sums (ACT: Exp) ----
            or_sb = op_.tile([M80, VB * 3], f32, tag="orsb")
            for v in range(VB):
                racc = pr.tile([M80, 3], f32, tag="racc")
                for c in range(NPC):
                    sl = slice(c * PCH, (c + 1) * PCH)
                    col = v * NPC + c
                    st_ = ps.tile([PCH, M80], f32, tag="st")
                    nc.tensor.matmul(st_, vts[2 * v][:, sl], w0,
                                     start=True, stop=False)
                    nc.tensor.matmul(st_, vts[2 * v + 1][:, sl], w1,
                                     start=False, stop=True)
                    et = ep.tile([PCH, M80], bf16, tag="et")
                    nc.scalar.activation(et, st_, AFT.Exp,
                                         scale=rm10[:, col:col + 1])
                    nc.tensor.matmul(racc, et,
                                     msk_t[:, col * 3:col * 3 + 3],
                                     start=(c == 0), stop=(c == NPC - 1))
                nc.vector.tensor_copy(or_sb[:, v * 3:(v + 1) * 3], racc)
            nc.gpsimd.dma_start(d_or[:], or_sb)

    nc.compile()
    return nc


_MODULE = None


def _get_module():
    global _MODULE
    if _MODULE is None:
        _MODULE = _build_module()
    return _MODULE


def kernel(video_feats, query_feats, sents_feats, iou2d, iou2ds, num_targets):
    video_feats = np.ascontiguousarray(np.asarray(video_feats, np.float32))
    query_feats = np.asarray(query_feats, np.float32)
    sents_feats = np.asarray(sents_feats, np.float32)
    iou2d = np.asarray(iou2d, np.float32)
    iou2ds = np.asarray(iou2ds, np.float32)
    nt = np.asarray(num_targets)
    assert video_feats.shape == (B, C, D, D) and sents_feats.shape == (T, C)
    assert (nt == NPT).all(), "kernel assumes uniform num_targets == 2"

    rows, cols = np.triu_indices(D)
    tri_lin = rows * D + cols                          # (2080,) row-major

    vf_flat = video_feats.reshape(B, C, SP)
    iou_flat = iou2d.reshape(B, SP)
    iouf = iou2ds.reshape(T, SP)[:, tri_lin]           # (T, 2080)
    pstar = tri_lin[np.argmax(iouf, axis=1)]           # top-1 pos per sentence
    scatter = np.repeat(np.arange(B), NPT)
    tvr = vf_flat[scatter, :, pstar]                   # (T, C) raw topk feats

    # compact triu columns, pad to SPP, cast bf16
    v16 = np.zeros((B, C, SPP), BF)
    v16[:, :, :NTRIU] = vf_flat[:, :, tri_lin].astype(BF)

    qT = np.ascontiguousarray(query_feats.T)           # (C, B)
    sT = np.ascontiguousarray(sents_feats.T)           # (C, T)
    bd = np.zeros((B, T), np.float32)
    bd[scatter, np.arange(T)] = 1.0
    qrn = 1.0 / np.maximum(np.linalg.norm(query_feats, axis=1), 1e-12)
    trn_all = 1.0 / np.maximum(np.linalg.norm(tvr, axis=1), 1e-12)  # (T,)
    qn = query_feats * qrn[:, None]                    # (B, C) normalized
    tvn = tvr * trn_all[:, None]                       # (T, C) normalized

    iou_tri = iou_flat[:, tri_lin]                     # (B, 2080)

    in_maps = []
    for k in range(NCORES):
        g0 = k * VB
        tv_loc = tvr[NPT * g0: NPT * g0 + NTC]         # (16, C) raw
        smallf = np.concatenate([qT, tv_loc.T, sT], axis=1)       # (C, 208)
        w16 = np.concatenate([qn, tvn[NPT * g0: NPT * g0 + NTC]],
                             axis=0).T.astype(BF)                 # (C, 80)
        msk = np.zeros((VB, SPP, 3), np.float32)
        for v in range(VB):
            g = g0 + v
            msk[v, :NTRIU, 0] = 1.0
            msk[v, :NTRIU, 1] = iou_tri[g] > NEG_IOU
            msk[v, :NTRIU, 2] = iou_tri[g] < NEG_IOU
        mskr = msk.reshape(VB, NPC, PCH, 3).transpose(0, 2, 1, 3).reshape(
            VB * PCH, NPC * 3).astype(BF)
        in_maps.append({
            "v16": v16[g0:g0 + VB].reshape(VB * C, SPP),
            "w16": np.ascontiguousarray(w16),
            "sf": np.ascontiguousarray(smallf),
            "msk": np.ascontiguousarray(mskr),
            "bd": bd,
            "qrn": qrn[:, None].astype(np.float32),
            "trn": trn_all[NPT * g0: NPT * g0 + NTC, None].astype(np.float32),
        })

    nc = _get_module()
    res = bass_utils.run_bass_kernel_spmd(nc, in_maps, core_ids=list(range(NCORES)))
    kernel._last = res
    outs = res.results

    # ---- host finalization (tiny, float64) ----
    E = np.float64
    smalls = [o["o_small"].astype(E) for o in outs]

    # L2 denominators: o_r col 3v+0 = sum exp over valid triu positions
    negq = np.zeros(B)
    for k in range(NCORES):
        r = outs[k]["o_r"][:B].astype(E)               # (64, 24)
        negq += r[:, 0::3].sum(axis=1)
    for b in range(B):
        k, v = b // VB, b % VB
        negq[b] -= float(outs[k]["o_r"][b, 3 * v + 1])

    t1 = np.empty(T)
    t2 = np.empty(T)
    t4 = np.empty(T)
    sm0 = smalls[0]
    s4 = outs[0]["o_s4"].astype(E)
    for t in range(T):
        b = scatter[t]
        k, v = b // VB, b % VB
        lc = NPT * v + (t - NPT * b)
        pos = smalls[k][b, lc]
        asum = float(outs[k]["o_cs1"][0, lc])
        t1[t] = -(pos * TAU_I - np.log(asum))
        t2[t] = -(pos * TAU_I - np.log(np.exp(pos * TAU_I) + negq[b]))
        pos4 = sm0[b, NTC + t]
        ns4 = s4[b, 0] - s4[b, 1]
        t4[t] = -(pos4 * TAU_I - np.log(np.exp(pos4 * TAU_I) + ns4))

    t3 = []
    for g in range(B):
        k, v = g // VB, g % VB
        a3 = outs[k]["o_a3"].astype(E)
        r = outs[k]["o_r"].astype(E)
        for i in range(NPT):
            ns = r[B + NPT * v + i, 3 * v + 2]
            for j in range(NPT):
                pd = a3[NPT * v + i, NPT * v + j]
                t3.append(-(pd * TAU_I - np.log(np.exp(pd * TAU_I) + ns)))

    return np.stack([t1.mean(), t2.mean(), np.mean(t3), t4.mean()]).astype(np.float32)



# revision 4
# speedup vs baseline: 4.3006x; 4.3006x over previous
"""Trainium2 Bass kernel for a 4-term video/query contrastive loss.

Strategy (v2): data-parallel over batch B=64 across 8 cores (8 videos/core).
The only device-side work is the big cross-contrast term: every one of the
80 weight rows (64 queries + 16 local top-1 features) scored against all
8*2080 = 16640 upper-tri proposal features of the core's videos, exp'd at
1/temperature, and mask-reduced per (video, {valid, iou>0.5}).

  - host pre-normalizes everything; V ships as fp8e4 [C, 16640] (exact
    16640 = 130*128, no padding), W as fp8e4 [C, 80]
  - scores are computed TRANSPOSED, S^T[p, r] (proposals on partitions):
    weights = fp8 v-chunks (FWL 4x load), stream = W (N=80), accumulated
    over the two C-halves in PSUM; 6 chunks batched per PSUM bank
  - one Exp per 6-chunk group ([128, 480], scale=10) -> bf16
  - one mask matmul per group: lhsT = [128, 96] block mask (16 cols per
    chunk: valid/pos per video), rhs = et [128, 480], PSUM-accumulated
    across all 22 groups into a single [96, 480] tensor; the host sums
    the 6 diagonal [16, 80] blocks
  - everything else (top-k gather, pos-pair cosines, the three small
    loss terms, logs/means) is tiny and runs on the host in float64

Device HBM traffic/core: 4.26 MB video fp8 + 0.55 MB masks -> memory
roofline ~13 us; ACT (1.33M exps) ~15 us; PE ~13 us.
"""

import numpy as np
import ml_dtypes

import concourse.bacc as bacc
import concourse.bass as bass
import concourse.tile as tile
from concourse import mybir
from concourse import bass_utils

f32 = mybir.dt.float32
bf16 = mybir.dt.bfloat16
AFT = mybir.ActivationFunctionType
BF = ml_dtypes.bfloat16

DT = mybir.dt.float8e4
DTNP = ml_dtypes.float8_e4m3

B, C, D = 64, 256, 64
T = 128
P = 2080                    # upper-tri positions per video
NCORES = 8
VB = B // NCORES            # videos per core: 8
NPT = 2                     # sentences per video
NPROP = VB * P              # 16640 proposals per core = 130 * 128 exactly
PCH = 128                   # proposals per chunk (partition dim of S^T)
NCH = NPROP // PCH          # 130 chunks
G = 6                       # chunks per exp/mask group (6*80 = 480 f32 <= bank)
NG = (NCH + G - 1) // G     # 22 groups (21 full + 1 of 4)
NW = B + NPT * VB           # 80 score rows
MC = 2 * VB                 # 16 mask cols: (valid, iou>0.5) per local video
GM = G * MC                 # 96 block-mask cols per group
SLABCH = 24                 # chunks per DMA slab
SLABW = SLABCH * PCH        # 3072
NSLAB = (NCH + SLABCH - 1) // SLABCH   # 6 (5 full + 1 of 10 chunks)
TAU = 10.0
NEG_IOU = 0.5


def _build_module():
    nc = bacc.Bacc("TRN2", target_bir_lowering=False, debug=False)

    d_v = nc.dram_tensor("v8", (C, NPROP), DT, kind="ExternalInput")
    d_w = nc.dram_tensor("w8", (C, NW), DT, kind="ExternalInput")
    d_m = nc.dram_tensor("msk", (PCH, NG * GM), bf16, kind="ExternalInput")
    d_or = nc.dram_tensor("o_r", (GM, G * NW), f32, kind="ExternalOutput")

    SG = 2 * G                  # 12 chunks per super-group (2 PSUM banks)
    NSG = (NCH + SG - 1) // SG  # 11 super-groups (10 full + 1 of 10)
    BANKF = 512                 # f32 elements per PSUM bank

    with tile.TileContext(nc) as tc:
        with (
            tc.tile_pool(name="consts", bufs=1) as cp,
            tc.tile_pool(name="vsl", bufs=3) as vp,
            tc.tile_pool(name="ets", bufs=3) as ep,
            tc.tile_pool(name="outs", bufs=1) as op_,
            tc.tile_pool(name="ps", bufs=3, space="PSUM") as ps,
            tc.tile_pool(name="pr", bufs=1, space="PSUM") as pr,
        ):
            # w + first slab on the sync queue; second C-half of each slab on
            # the vector queue (parallel DMA stream); masks on gpsimd.
            w0 = cp.tile([128, NW], DT, tag="w0")
            w1 = cp.tile([128, NW], DT, tag="w1")
            nc.sync.dma_start(w0, d_w[0:128, :])
            nc.sync.dma_start(w1, d_w[128:256, :])
            mt = cp.tile([PCH, NG * GM], bf16, tag="mt")
            nc.gpsimd.dma_start(mt, d_m[:])

            vts = []
            for s in range(NSLAB):
                wd = min(SLABW, NPROP - s * SLABW)
                t0 = vp.tile([128, SLABW], DT, tag="v0")
                t1 = vp.tile([128, SLABW], DT, tag="v1")
                nc.sync.dma_start(t0[:, 0:wd], d_v[0:128, s * SLABW:s * SLABW + wd])
                nc.gpsimd.dma_start(t1[:, 0:wd], d_v[128:256, s * SLABW:s * SLABW + wd])
                vts.append((t0, t1))

            rsum = pr.tile([GM, G * NW], f32, tag="rs")
            ets = []  # et3 tile per super-group

            def mask_mm(g):
                sg, jj = divmod(g, 2)
                nc.tensor.matmul(rsum, mt[:, g * GM:(g + 1) * GM],
                                 ets[sg][:, jj, :],
                                 start=(g == 0), stop=(g == NG - 1))

            for sg in range(NSG):
                cg = min(SG, NCH - sg * SG)
                st = ps.tile([128, 2, BANKF], f32, tag="st")
                for j in range(cg):
                    c = sg * SG + j
                    s, off = divmod(c, SLABCH)
                    jj, m = divmod(j, G)
                    t0, t1 = vts[s]
                    sl = slice(off * PCH, (off + 1) * PCH)
                    ds = st[:, jj, m * NW:(m + 1) * NW]
                    nc.tensor.matmul(ds, t0[:, sl], w0,
                                     start=(m == 0), stop=False)
                    nc.tensor.matmul(ds, t1[:, sl], w1,
                                     start=False, stop=(m == G - 1 or j == cg - 1))
                et = ep.tile([128, 2, G * NW], bf16, tag="et")
                if cg < SG:
                    nc.vector.memset(et, 0.0)
                    nc.scalar.activation(et[:, 0, :], st[:, 0, 0:G * NW],
                                         AFT.Exp, scale=TAU)
                    r = cg - G
                    nc.scalar.activation(et[:, 1, 0:r * NW], st[:, 1, 0:r * NW],
                                         AFT.Exp, scale=TAU)
                else:
                    nc.scalar.activation(et[:, :, :], st[:, :, 0:G * NW],
                                         AFT.Exp, scale=TAU)
                ets.append(et)
                if sg >= 1:
                    mask_mm(2 * (sg - 1))
                    mask_mm(2 * sg - 1)
            mask_mm(2 * NSG - 2)
            mask_mm(2 * NSG - 1)

            rs_sb = op_.tile([GM, G * NW], f32, tag="rsb")
            nc.vector.tensor_copy(rs_sb, rsum)
            nc.sync.dma_start(d_or[:], rs_sb)

    nc.compile()
    return nc


_MODULE = None


def _get_module():
    global _MODULE
    if _MODULE is None:
        _MODULE = _build_module()
    return _MODULE


def kernel(video_feats, query_feats, sents_feats, iou2d, iou2ds, num_targets):
    video_feats = np.ascontiguousarray(np.asarray(video_feats, np.float32))
    query_feats = np.asarray(query_feats, np.float32)
    sents_feats = np.asarray(sents_feats, np.float32)
    iou2d = np.asarray(iou2d, np.float32)
    iou2ds = np.asarray(iou2ds, np.float32)
    nt = np.asarray(num_targets)
    assert video_feats.shape == (B, C, D, D) and sents_feats.shape == (T, C)
    assert (nt == NPT).all(), "kernel assumes uniform num_targets == 2"

    rows, cols = np.triu_indices(D)
    tri = rows * D + cols

    vf = video_feats.reshape(B, C, D * D)[:, :, tri]           # (64, 256, 2080)
    nrm = np.sqrt(np.einsum('bcp,bcp->bp', vf, vf))
    vhat = vf / np.maximum(nrm, 1e-12)[:, None, :]

    qn = query_feats / np.maximum(
        np.linalg.norm(query_feats, axis=1, keepdims=True), 1e-12)
    sn = sents_feats / np.maximum(
        np.linalg.norm(sents_feats, axis=1, keepdims=True), 1e-12)

    iouf = iou2ds.reshape(T, D * D)[:, tri]
    pstar = iouf.argmax(1)                                     # top-1 per sentence
    scatter = np.repeat(np.arange(B), NPT)
    tvn = vhat[scatter, :, pstar]                              # (128, 256) normalized
    iou_tri = iou2d.reshape(B, D * D)[:, tri]
    posm_all = iou_tri > NEG_IOU

    vidx = np.repeat(np.arange(VB), P)
    ar = np.arange(NPROP)
    in_maps = []
    for k in range(NCORES):
        g0 = VB * k
        vcat = np.ascontiguousarray(
            vhat[g0:g0 + VB].transpose(1, 0, 2).reshape(C, NPROP)).astype(DTNP)
        wk = np.ascontiguousarray(np.concatenate(
            [qn, tvn[2 * g0:2 * g0 + 2 * VB]], 0).T).astype(DTNP)   # (256, 80)
        m = np.zeros((NG * G * PCH, MC), np.float32)
        pos = posm_all[g0:g0 + VB].reshape(-1)
        m[ar, 2 * vidx] = 1.0
        m[ar, 2 * vidx + 1] = pos
        mh = m.reshape(NG, G, PCH, MC).transpose(2, 0, 1, 3).reshape(
            PCH, NG * GM).astype(BF)
        in_maps.append({
            "v8": vcat,
            "w8": wk,
            "msk": np.ascontiguousarray(mh),
        })

    nc = _get_module()
    res = bass_utils.run_bass_kernel_spmd(nc, in_maps, core_ids=list(range(NCORES)))
    kernel._last = res
    outs = res.results

    # ---- host finalization (tiny, float64) ----
    E = np.float64
    valid = np.zeros((NCORES, VB, NW))
    posv = np.zeros((NCORES, VB, NW))
    for k in range(NCORES):
        rs = outs[k]["o_r"].astype(E)                          # (96, 480)
        acc = np.zeros((MC, NW))
        for j in range(G):
            acc += rs[MC * j:MC * (j + 1), NW * j:NW * (j + 1)]
        valid[k] = acc[0::2, :]
        posv[k] = acc[1::2, :]

    tvn64, qn64, sn64 = tvn.astype(E), qn.astype(E), sn.astype(E)
    negq = valid[:, :, :B].sum(axis=(0, 1))                    # (64,)
    for b in range(B):
        negq[b] -= posv[b // VB, b % VB, b]

    pos_t = (tvn64 * qn64[scatter]).sum(1)                     # (128,)
    E1 = np.exp(TAU * qn64 @ tvn64.T)                          # (64, 128)
    asum = E1.sum(0)
    t1 = -(TAU * pos_t - np.log(asum))
    t2 = -(TAU * pos_t - np.log(np.exp(TAU * pos_t) + negq[scatter]))

    a3 = tvn64 @ tvn64.T
    t3 = []
    for g in range(B):
        k, v = g // VB, g % VB
        for i in (NPT * g, NPT * g + 1):
            r = B + (i - 2 * VB * k)
            neg_i = valid[k, v, r] - posv[k, v, r]
            for j in (NPT * g, NPT * g + 1):
                pd = a3[i, j]
                t3.append(-(TAU * pd - np.log(np.exp(TAU * pd) + neg_i)))

    pos4 = (sn64 * qn64[scatter]).sum(1)
    E4 = np.exp(TAU * qn64 @ sn64.T)                           # (64, 128)
    mask4 = (scatter[None, :] != np.arange(B)[:, None])
    negsum4 = (E4 * mask4).sum(1)
    t4 = -(TAU * pos4 - np.log(np.exp(TAU * pos4) + negsum4[scatter]))

    return np.stack([t1.mean(), t2.mean(), np.mean(t3), t4.mean()]).astype(np.float32)
